# revision 18
# baseline (speedup 1.0000x reference)
"""CTC forward-loss kernel for 8 Trainium2 NeuronCores (Bass/Tile).

Segment-major wavefront layout: partition p = 8*s + b (s = time segment
0..15, b = local batch row 0..7).  Wave w processes cell (u = w - 3s, s)
on every partition.  Per column u the CTC recurrence runs as a Viterbi
(max-plus) sweep (VE/W scans) plus an exp-domain sum sweep (pE/q scans)
normalized by the Viterbi values; the exp sweep trails by LAG waves.

v2 engine split (DVE holds ONLY the four scans + one guard copy):
  PE  : one merged [128,4] carry matmul per wave (shift-by-8 partitions);
        Q/PE ring slots are LAG-shifted so V and E carries share a slot.
  DVE : VE scan, W scan, pE scan, q scan, W-guard copy.
  Pool: D = W[t-1]-W[t]; Aarg|AOarg = (em|blk)+D; DLarg = Wprev-W;
        FEED = DL * q_prev (emitted one trace early).
  ACT : exp(Aarg|AOarg), exp(DLarg + c2s), carry-init (q,pE guards) =
        PSUM * exp(-jmp) + init, readouts.
The exp-sweep carries are PRE-SCALED by exp(-jmp) when crossing segment
bases, which eliminates the old per-wave elem0 fixups entirely (the
boundary exp-args then need exactly the raw guard values the main slab
ops already produce).
"""

import os
import sys
import numpy as np

T, B, C, S = 1000, 64, 28, 200
NCORES = 8
BC = B // NCORES            # batch rows per core
L = 64                      # t-segment length
NSEG = 16                   # segments (15*64 + 40, padded to 16*64)
SLAG = int(os.environ.get("CTC_SLAG", "3"))  # wavefront lag per segment step
NW = (S - 1) + SLAG * (NSEG - 1) + 1   # wave slots
WD = 8                      # ring depth (slots) for state rings
SL = L + 1                  # slot length: guard elem + L payload
RS = WD * SL                # ring row size
VD = 4                      # exp-arg ring depth
LAG = int(os.environ.get("CTC_LAG", "3"))  # exp sweep trails V sweep
C1, C2, OFF = 0.28, 1.3, 15.0
NEG = -1.0e30
SMALL_NEG = -1.0e4          # emission for invalid (u out of range) cells


def _rho(s, u):
    tmid = min(s * L + L // 2, T - 1)
    return np.float32(min(C1 * tmid, C2 * u) + OFF)


def host_prep(prediction, target):
    """Per-core input planes for the segment-major wave layout."""
    pred = np.asarray(prediction, dtype=np.float32)
    tgt = np.maximum(np.asarray(target).astype(np.int64) - 1, 0)
    emitE = np.take_along_axis(
        pred, np.broadcast_to(tgt[None], (T, B, S)), axis=2
    ).astype(np.float32)                      # [T,B,S]
    blank = pred[:, :, C - 1].astype(np.float32)   # [T,B]

    TP = NSEG * L
    emitP = np.zeros((TP, B, S), np.float32); emitP[:T] = emitE
    blankP = np.zeros((TP, B), np.float32); blankP[:T] = blank

    jmp = np.zeros((128, NW), np.float32)
    c2s = np.full((128, NW), NEG, np.float32)  # NEG kills DL at invalid cells
    for s in range(NSEG):
        p0, p1 = s * BC, (s + 1) * BC
        for w in range(NW):
            u = w - SLAG * s
            if not (0 <= u < S):
                continue
            jmp[p0:p1, w] = _rho(s, u) - (_rho(s - 1, u) if s > 0 else 0.0)
            c2s[p0:p1, w] = (_rho(s, u - 1) - _rho(s, u)) if u >= 1 else 0.0
            # (u==0 keeps c2s=0; its DL dies via the SMALL_NEG Wprev column)

    with np.errstate(over="ignore", under="ignore"):
        ejmpn = np.exp(-jmp.astype(np.float64)).astype(np.float32)
    # exp-sweep init mass lives in the rho base: at segment 0 the virtual
    # q/pE start is exp(-rho(0,u)) = ejmpn (jmp[s=0] = rho(0,u) - 0).
    initPw = np.zeros((128, NW), np.float32)
    initPw[0:BC] = ejmpn[0:BC]
    shift8 = np.zeros((128, 128), np.float32)
    for j in range(BC, 128):
        shift8[j - BC, j] = 1.0

    ins = []
    for c in range(NCORES):
        em = np.full((NSEG, BC, NW + 1, L), SMALL_NEG, np.float32)
        for s in range(NSEG):
            em[s, :, SLAG * s:SLAG * s + S, :] = emitP[s * L:(s + 1) * L,
                                         c * BC:(c + 1) * BC, :].transpose(1, 2, 0)
            em[s, :, NW, :] = blankP[s * L:(s + 1) * L,
                                     c * BC:(c + 1) * BC].T
        ins.append({
            "emplane": np.ascontiguousarray(em.reshape(128, (NW + 1) * L)),
            "c2s": c2s, "ejmpn": ejmpn,
            "initPw": initPw, "shift8": shift8,
        })
    return ins


def readout_cells(pl, tl):
    """(g, u_e, t_b, s_b, partition, wave, elem) per batch row."""
    out = []
    for g in range(B):
        t_b = int(pl[g]) - 1
        u_e = int(tl[g]) - 1
        s_b = t_b // L
        p = s_b * BC + (g % BC)
        out.append((g, u_e, t_b, s_b, p, u_e + SLAG * s_b, t_b - s_b * L))
    return out


def build_kernel_body(tc, outs, ins, rcells):
    import concourse.tile as tile  # noqa: F401
    from concourse import mybir
    from contextlib import ExitStack

    nc = tc.nc
    f32 = mybir.dt.float32
    Alu = mybir.AluOpType
    Act = mybir.ActivationFunctionType

    ctx = ExitStack()
    planes = ctx.enter_context(tc.tile_pool(name="planes", bufs=1))
    psum = ctx.enter_context(tc.tile_pool(name="psum", bufs=1, space="PSUM"))

    emplane = planes.tile([128, NW + 1, L], f32)
    c2s = planes.tile([128, NW], f32)
    ejmpn = planes.tile([128, NW], f32)
    initPw = planes.tile([128, NW], f32)
    shift8 = planes.tile([128, 128], f32)

    NCH = 16
    em2d = emplane.rearrange("p a b -> p (a b)")
    tot = (NW + 1) * L
    chunk = (tot + NCH - 1) // NCH
    for i in range(NCH):
        lo, hi = i * chunk, min((i + 1) * chunk, tot)
        nc.sync.dma_start(out=em2d[:, lo:hi], in_=ins["emplane"][:, lo:hi])
    for t_sb, t_dr in ((c2s, ins["c2s"]), (ejmpn, ins["ejmpn"]),
                       (initPw, ins["initPw"]), (shift8, ins["shift8"])):
        nc.sync.dma_start(out=t_sb, in_=t_dr)

    blk = emplane[:, NW, :]

    # state rings: rows 0=W 1=VE 2=Q 3=PE, each [RS] with guard at slot*SL.
    # Q/PE slots are LAG-shifted: E wave w2 lives at slot (w2+LAG) % WD.
    STATE = planes.tile([128, 4, RS], f32)
    nc.vector.memset(STATE[:, 0, :], NEG)
    nc.vector.memset(STATE[:, 1, :], NEG)
    nc.vector.memset(STATE[:, 2, :], 0.0)
    nc.vector.memset(STATE[:, 3, :], 0.0)

    EXPARG = planes.tile([128, VD, 3 * L], f32)
    AEXP = planes.tile([128, VD, 3 * L], f32)
    Dring = planes.tile([128, 2, L], f32)
    FEEDr = planes.tile([128, 2, L], f32)
    outWs = planes.tile([128, B], f32)
    outPs = planes.tile([128, B], f32)
    zeros1 = planes.tile([128, 1], f32)
    nc.vector.memset(outWs, 0.0)
    nc.vector.memset(outPs, 0.0)
    nc.vector.memset(zeros1, 0.0)

    # merged carry PSUM: cols 0=Wc 1=VEc 2=Qc 3=PEc, ring of 4 traces
    CAR4 = psum.tile([128, 4, 4], f32)

    rd_by_wave_W = {}
    rd_by_wave_P = {}
    for (g, u_e, t_b, s_b, p, wave, elem) in rcells:
        rd_by_wave_W.setdefault(wave, []).append((g, p, elem))
        rd_by_wave_P.setdefault(wave, []).append((g, p, elem))

    limit = NW

    def emit_mm(tn):
        # one [128,4] carry matmul: V rings at slot (tn-SLAG)%WD; E rings
        # occupy the same slot index thanks to the LAG shift.
        ks = ((tn - SLAG) % WD) * SL
        nc.tensor.matmul(
            CAR4[:, tn % 4, :], shift8,
            STATE[:, 0:4, ks + L:ks + L + 1].rearrange("p a b -> p (a b)"),
            start=True, stop=True)

    emit_mm(0)
    emit_mm(1)
    # V guards for wave 0 (W + VE carries into slot-0 SBUF guards)
    nc.vector.tensor_copy(
        out=STATE[:, 0:2, 0:1].rearrange("p a b -> p (a b)"),
        in_=CAR4[:, 0, 0:2])
    for tw in range(limit + LAG):
        w = tw
        w2 = tw - LAG
        wv = tw - 1
        cv = CAR4[:, tw % 4, :]
        k = (w % WD) * SL
        kp = ((w - 1) % WD) * SL

        # ---- carry matmul two traces ahead (SLAG>=2 keeps inputs past) ----
        if tw + 2 < limit + LAG:
            emit_mm(tw + 2)

        # ---- carry-init ACT for E wave tw-LAG+2, right behind its matmul
        # so it sits at the head of the ACT queue (pE/q scans consume it
        # next trace; keeping this early removes the DVE stall).
        iw2 = tw - LAG + 2
        if 0 <= iw2 < limit:
            k2n = ((iw2 + LAG) % WD) * SL
            # q guard (row 2) and pE init (row 3) = carry*exp(-jmp) + init
            nc.scalar.activation(
                out=STATE[:, 2:4, k2n:k2n + 1].rearrange("p a b -> p (a b)"),
                in_=CAR4[:, (iw2 + LAG) % 4, 2:4],
                func=Act.Identity,
                bias=initPw[:, iw2:iw2 + 1],
                scale=ejmpn[:, iw2:iw2 + 1])

        # ---- V/E scans interleaved: VE(w), pE(w2), W(w), q(w2) so no
        # DVE op reads the immediately preceding op's fresh output ----
        k2 = ((w2 + LAG) % WD) * SL
        if tw < limit:
            # VE[t] = max(Wprev[t-1], VE[t-1]) + em[t]
            nc.vector.tensor_tensor_scan(
                STATE[:, 1, k + 1:k + L + 1],
                STATE[:, 0, kp:kp + L],
                emplane[:, w, :],
                STATE[:, 1, k:k + 1], Alu.max, Alu.add)
        if tw >= LAG:
            # pE[t] = (FEED[t] + pE[t-1]) * A[t]
            nc.vector.tensor_tensor_scan(
                STATE[:, 3, k2 + 1:k2 + L + 1],
                FEEDr[:, w2 % 2, :],
                AEXP[:, w2 % VD, 0:L],
                STATE[:, 3, k2:k2 + 1], Alu.add, Alu.mult)
        if tw < limit:
            # W[t] = max(blk[t] + W[t-1], VE[t])
            nc.vector.tensor_tensor_scan(
                STATE[:, 0, k + 1:k + L + 1],
                blk,
                STATE[:, 1, k + 1:k + L + 1],
                STATE[:, 0, k:k + 1], Alu.add, Alu.max)
        if tw >= LAG:
            # q[t] = AO[t] * q[t-1] + pE[t]
            nc.vector.tensor_tensor_scan(
                STATE[:, 2, k2 + 1:k2 + L + 1],
                AEXP[:, w2 % VD, L:2 * L],
                STATE[:, 3, k2 + 1:k2 + L + 1],
                STATE[:, 2, k2:k2 + 1], Alu.mult, Alu.add)

        # ---- V guard copy for wave w+1 (W + VE carries -> SBUF guards,
        # one trace early so next trace's scans read SBUF, not PSUM) ----
        if tw + 1 < limit:
            k1 = ((tw + 1) % WD) * SL
            nc.vector.tensor_copy(
                out=STATE[:, 0:2, k1:k1 + 1].rearrange("p a b -> p (a b)"),
                in_=CAR4[:, (tw + 1) % 4, 0:2])

        # ---- V slabs for wave wv = tw - 1 ----
        if 0 <= wv < limit:
            kv, kvp = (wv % WD) * SL, ((wv - 1) % WD) * SL
            # D[t] = W[t-1] - W[t]  (elem0 = carry guard - W[0], correct)
            nc.gpsimd.tensor_tensor(out=Dring[:, wv % 2, :],
                                    in0=STATE[:, 0, kv:kv + L],
                                    in1=STATE[:, 0, kv + 1:kv + L + 1],
                                    op=Alu.subtract)
            # Aarg|AOarg = (em|blk) + D
            nc.gpsimd.tensor_tensor(
                out=EXPARG[:, wv % VD, 0:2 * L].rearrange("p (a b) -> p a b", a=2),
                in0=emplane[:, wv::(NW - wv), :],
                in1=Dring[:, wv % 2, :].unsqueeze(1).broadcast_to([128, 2, L]),
                op=Alu.add)
            # DLarg = Wprev[t-1] - W[t-1]  (elem0 = guard' - guard, correct)
            nc.gpsimd.tensor_tensor(
                out=EXPARG[:, wv % VD, 2 * L:3 * L],
                in0=STATE[:, 0, kvp:kvp + L],
                in1=STATE[:, 0, kv:kv + L], op=Alu.subtract)
            nc.scalar.activation(out=AEXP[:, wv % VD, 0:2 * L],
                                 in_=EXPARG[:, wv % VD, 0:2 * L], func=Act.Exp)
            nc.scalar.activation(out=AEXP[:, wv % VD, 2 * L:3 * L],
                                 in_=EXPARG[:, wv % VD, 2 * L:3 * L], func=Act.Exp,
                                 bias=c2s[:, wv:wv + 1])

        # ---- E readouts: wave w2 ----
        if tw >= LAG:
            for (g, p, elem) in rd_by_wave_P.get(w2, ()):
                q = (p // 32) * 32
                nc.gpsimd.tensor_tensor(
                    out=outPs[q:q + 32, g:g + 1],
                    in0=STATE[q:q + 32, 3, k2 + 1 + elem:k2 + 2 + elem],
                    in1=zeros1[q:q + 32, 0:1], op=Alu.add)

        # ---- FEED for E wave nw2 = w2+1 = DL[nw2] * q[nw2-1] ring ----
        # AEXP[nw2] was produced by this trace's slab block (wv == nw2);
        # emitted AFTER the q scan so Tile sees the fresh q values.
        nw2 = tw - LAG + 1
        if 0 <= nw2 < limit:
            kq = ((nw2 - 1 + LAG) % WD) * SL
            nc.gpsimd.tensor_tensor(
                out=FEEDr[:, nw2 % 2, :],
                in0=AEXP[:, nw2 % VD, 2 * L:3 * L],
                in1=STATE[:, 2, kq:kq + L],
                op=Alu.mult)

        if tw < limit:
            for (g, p, elem) in rd_by_wave_W.get(w, ()):
                q = (p // 32) * 32
                nc.gpsimd.tensor_tensor(
                    out=outWs[q:q + 32, g:g + 1],
                    in0=STATE[q:q + 32, 0, k + 1 + elem:k + 2 + elem],
                    in1=zeros1[q:q + 32, 0:1], op=Alu.add)

    nc.sync.dma_start(out=outs["outW"], in_=outWs)
    nc.sync.dma_start(out=outs["outP"], in_=outPs)
    ctx.close()


def _build_program(rcells):
    import concourse.bacc as bacc
    import concourse.tile as tile_mod
    from concourse import mybir

    nc = bacc.Bacc("TRN2", target_bir_lowering=False, debug=False,
                   num_devices=NCORES)
    f32 = mybir.dt.float32
    ins = {
        "emplane": nc.declare_dram_parameter("emplane", [128, (NW + 1) * L], f32,
                                             isOutput=False).ap(),
        "c2s": nc.declare_dram_parameter("c2s", [128, NW], f32, isOutput=False).ap(),
        "ejmpn": nc.declare_dram_parameter("ejmpn", [128, NW], f32, isOutput=False).ap(),
        "initPw": nc.declare_dram_parameter("initPw", [128, NW], f32, isOutput=False).ap(),
        "shift8": nc.declare_dram_parameter("shift8", [128, 128], f32, isOutput=False).ap(),
    }
    outs = {
        "outW": nc.declare_dram_parameter("outW", [128, B], f32, isOutput=True).ap(),
        "outP": nc.declare_dram_parameter("outP", [128, B], f32, isOutput=True).ap(),
    }
    with tile_mod.TileContext(nc) as tc:
        build_kernel_body(tc, outs, ins, rcells)
    nc.compile()
    return nc


def kernel(prediction, target, pred_lens, target_lens):
    sys.path.insert(0, "/opt/trn_rl_repo")
    from concourse.bass_utils import run_bass_kernel_spmd

    pl = np.asarray(pred_lens).astype(np.int64)
    tl = np.asarray(target_lens).astype(np.int64)
    rcells = readout_cells(pl, tl)
    in_maps = host_prep(prediction, target)

    nc = _build_program(rcells)
    res = run_bass_kernel_spmd(nc, in_maps, list(range(NCORES)))
    global LAST_RESULTS
    LAST_RESULTS = res

    total = 0.0
    for (g, u_e, t_b, s_b, p, wave, elem) in rcells:
        c = g // BC
        pv = np.float64(res.results[c]["outP"][p, g])
        wv = np.float64(res.results[c]["outW"][p, g])
        total += np.log(pv) + wv + float(_rho(s_b, u_e))
    return np.float32(total)


# revision 19
# speedup vs baseline: 1.1493x; 1.1493x over previous
"""CTC forward-loss kernel for 8 Trainium2 NeuronCores (Bass/Tile).

Segment-major wavefront layout: partition p = 8*s + b (s = time segment
0..15, b = local batch row 0..7).  Wave w processes cell (u = w - 3s, s)
on every partition.  Per column u the CTC recurrence runs as a Viterbi
(max-plus) sweep (VE/W scans) plus an exp-domain sum sweep (pE/q scans)
normalized by the Viterbi values; the exp sweep trails by LAG waves.

v2 engine split (DVE holds ONLY the four scans + one guard copy):
  PE  : one merged [128,4] carry matmul per wave (shift-by-8 partitions);
        Q/PE ring slots are LAG-shifted so V and E carries share a slot.
  DVE : VE scan, W scan, pE scan, q scan, W-guard copy.
  Pool: D = W[t-1]-W[t]; Aarg|AOarg = (em|blk)+D; DLarg = Wprev-W;
        FEED = DL * q_prev (emitted one trace early).
  ACT : exp(Aarg|AOarg), exp(DLarg + c2s), carry-init (q,pE guards) =
        PSUM * exp(-jmp) + init, readouts.
The exp-sweep carries are PRE-SCALED by exp(-jmp) when crossing segment
bases, which eliminates the old per-wave elem0 fixups entirely (the
boundary exp-args then need exactly the raw guard values the main slab
ops already produce).
"""

import os
import sys
import numpy as np

T, B, C, S = 1000, 64, 28, 200
NCORES = 8
BC = B // NCORES            # batch rows per core
L = 64                      # t-segment length
NSEG = 16                   # segments (15*64 + 40, padded to 16*64)
SLAG = int(os.environ.get("CTC_SLAG", "3"))  # wavefront lag per segment step
NW = (S - 1) + SLAG * (NSEG - 1) + 1   # wave slots
WD = 8                      # ring depth (slots) for state rings
SL = L + 1                  # slot length: guard elem + L payload
RS = WD * SL                # ring row size
VD = 4                      # exp-arg ring depth
LAG = int(os.environ.get("CTC_LAG", "3"))  # exp sweep trails V sweep
C1, C2, OFF = 0.28, 1.3, 15.0
NEG = -1.0e30
SMALL_NEG = -1.0e4          # emission for invalid (u out of range) cells


def _rho(s, u):
    tmid = min(s * L + L // 2, T - 1)
    return np.float32(min(C1 * tmid, C2 * u) + OFF)


def host_prep(prediction, target):
    """Per-core input planes for the segment-major wave layout."""
    pred = np.asarray(prediction, dtype=np.float32)
    tgt = np.maximum(np.asarray(target).astype(np.int64) - 1, 0)
    emitE = np.take_along_axis(
        pred, np.broadcast_to(tgt[None], (T, B, S)), axis=2
    ).astype(np.float32)                      # [T,B,S]
    blank = pred[:, :, C - 1].astype(np.float32)   # [T,B]

    TP = NSEG * L
    emitP = np.zeros((TP, B, S), np.float32); emitP[:T] = emitE
    blankP = np.zeros((TP, B), np.float32); blankP[:T] = blank

    jmp = np.zeros((128, NW), np.float32)
    c2s = np.full((128, NW), NEG, np.float32)  # NEG kills DL at invalid cells
    for s in range(NSEG):
        p0, p1 = s * BC, (s + 1) * BC
        for w in range(NW):
            u = w - SLAG * s
            if not (0 <= u < S):
                continue
            jmp[p0:p1, w] = _rho(s, u) - (_rho(s - 1, u) if s > 0 else 0.0)
            c2s[p0:p1, w] = (_rho(s, u - 1) - _rho(s, u)) if u >= 1 else 0.0
            # (u==0 keeps c2s=0; its DL dies via the SMALL_NEG Wprev column)

    with np.errstate(over="ignore", under="ignore"):
        ejmpn = np.exp(-jmp.astype(np.float64)).astype(np.float32)
    # exp-sweep init mass lives in the rho base: at segment 0 the virtual
    # q/pE start is exp(-rho(0,u)) = ejmpn (jmp[s=0] = rho(0,u) - 0).
    initPw = np.zeros((128, NW), np.float32)
    initPw[0:BC] = ejmpn[0:BC]
    shift8 = np.zeros((128, 128), np.float32)
    for j in range(BC, 128):
        shift8[j - BC, j] = 1.0

    ins = []
    for c in range(NCORES):
        em = np.full((NSEG, BC, NW + 1, L), SMALL_NEG, np.float32)
        for s in range(NSEG):
            em[s, :, SLAG * s:SLAG * s + S, :] = emitP[s * L:(s + 1) * L,
                                         c * BC:(c + 1) * BC, :].transpose(1, 2, 0)
            em[s, :, NW, :] = blankP[s * L:(s + 1) * L,
                                     c * BC:(c + 1) * BC].T
        ins.append({
            "emplane": np.ascontiguousarray(em.reshape(128, (NW + 1) * L)),
            "c2s": c2s, "ejmpn": ejmpn,
            "initPw": initPw, "shift8": shift8,
        })
    return ins


def readout_cells(pl, tl):
    """(g, u_e, t_b, s_b, partition, wave, elem) per batch row."""
    out = []
    for g in range(B):
        t_b = int(pl[g]) - 1
        u_e = int(tl[g]) - 1
        s_b = t_b // L
        p = s_b * BC + (g % BC)
        out.append((g, u_e, t_b, s_b, p, u_e + SLAG * s_b, t_b - s_b * L))
    return out


def build_kernel_body(tc, outs, ins, rcells):
    import concourse.tile as tile  # noqa: F401
    from concourse import mybir
    from contextlib import ExitStack

    nc = tc.nc
    f32 = mybir.dt.float32
    Alu = mybir.AluOpType
    Act = mybir.ActivationFunctionType

    ctx = ExitStack()
    planes = ctx.enter_context(tc.tile_pool(name="planes", bufs=1))
    psum = ctx.enter_context(tc.tile_pool(name="psum", bufs=1, space="PSUM"))

    emplane = planes.tile([128, NW + 1, L], f32)
    c2s = planes.tile([128, NW], f32)
    ejmpn = planes.tile([128, NW], f32)
    initPw = planes.tile([128, NW], f32)
    shift8 = planes.tile([128, 128], f32)

    NCH = 16
    em2d = emplane.rearrange("p a b -> p (a b)")
    tot = (NW + 1) * L
    chunk = (tot + NCH - 1) // NCH
    for i in range(NCH):
        lo, hi = i * chunk, min((i + 1) * chunk, tot)
        nc.sync.dma_start(out=em2d[:, lo:hi], in_=ins["emplane"][:, lo:hi])
    for t_sb, t_dr in ((c2s, ins["c2s"]), (ejmpn, ins["ejmpn"]),
                       (initPw, ins["initPw"]), (shift8, ins["shift8"])):
        nc.sync.dma_start(out=t_sb, in_=t_dr)

    blk = emplane[:, NW, :]

    # state rings: rows 0=W 1=VE 2=Q 3=PE, each [RS] with guard at slot*SL.
    # Q/PE slots are LAG-shifted: E wave w2 lives at slot (w2+LAG) % WD.
    STATE = planes.tile([128, 4, RS], f32)
    nc.vector.memset(STATE[:, 0, :], NEG)
    nc.vector.memset(STATE[:, 1, :], NEG)
    nc.vector.memset(STATE[:, 2, :], 0.0)
    nc.vector.memset(STATE[:, 3, :], 0.0)

    EXPARG = planes.tile([128, VD, 3 * L], f32)
    AEXP = planes.tile([128, VD, 3 * L], f32)
    Dring = planes.tile([128, 2, L], f32)
    FEEDr = planes.tile([128, 2, L], f32)
    outWs = planes.tile([128, B], f32)
    outPs = planes.tile([128, B], f32)
    zeros1 = planes.tile([128, 1], f32)
    nc.vector.memset(outWs, 0.0)
    nc.vector.memset(outPs, 0.0)
    nc.vector.memset(zeros1, 0.0)

    # merged carry PSUM: cols 0=Wc 1=VEc 2=Qc 3=PEc, ring of 4 traces
    CAR4 = psum.tile([128, 4, 4], f32)

    rd_by_wave_W = {}
    rd_by_wave_P = {}
    for (g, u_e, t_b, s_b, p, wave, elem) in rcells:
        rd_by_wave_W.setdefault(wave, []).append((g, p, elem))
        rd_by_wave_P.setdefault(wave, []).append((g, p, elem))

    limit = NW

    def emit_mm(tn):
        # one [128,4] carry matmul: V rings at slot (tn-SLAG)%WD; E rings
        # occupy the same slot index thanks to the LAG shift.
        ks = ((tn - SLAG) % WD) * SL
        nc.tensor.matmul(
            CAR4[:, tn % 4, :], shift8,
            STATE[:, 0:4, ks + L:ks + L + 1].rearrange("p a b -> p (a b)"),
            start=True, stop=True)

    emit_mm(0)
    emit_mm(1)
    # V guards for waves 0 and 1 (W + VE carries into SBUF guards)
    nc.vector.tensor_copy(
        out=STATE[:, 0:2, 0:1].rearrange("p a b -> p (a b)"),
        in_=CAR4[:, 0, 0:2])
    nc.vector.tensor_copy(
        out=STATE[:, 0:2, SL:SL + 1].rearrange("p a b -> p (a b)"),
        in_=CAR4[:, 1, 0:2])
    for tw in range(limit + LAG):
        w = tw
        w2 = tw - LAG
        wv = tw - 1
        cv = CAR4[:, tw % 4, :]
        k = (w % WD) * SL
        kp = ((w - 1) % WD) * SL

        # ---- V/E scans interleaved: VE(w), pE(w2), W(w), q(w2) so no
        # DVE op reads the immediately preceding op's fresh output ----
        k2 = ((w2 + LAG) % WD) * SL
        if tw < limit:
            # VE[t] = max(Wprev[t-1], VE[t-1]) + em[t]
            nc.vector.tensor_tensor_scan(
                STATE[:, 1, k + 1:k + L + 1],
                STATE[:, 0, kp:kp + L],
                emplane[:, w, :],
                STATE[:, 1, k:k + 1], Alu.max, Alu.add)
        if tw >= LAG:
            # pE[t] = (FEED[t] + pE[t-1]) * A[t]
            nc.vector.tensor_tensor_scan(
                STATE[:, 3, k2 + 1:k2 + L + 1],
                FEEDr[:, w2 % 2, :],
                AEXP[:, w2 % VD, 0:L],
                STATE[:, 3, k2:k2 + 1], Alu.add, Alu.mult)
        if tw < limit:
            # W[t] = max(blk[t] + W[t-1], VE[t])
            nc.vector.tensor_tensor_scan(
                STATE[:, 0, k + 1:k + L + 1],
                blk,
                STATE[:, 1, k + 1:k + L + 1],
                STATE[:, 0, k:k + 1], Alu.add, Alu.max)
        if tw >= LAG:
            # q[t] = AO[t] * q[t-1] + pE[t]
            nc.vector.tensor_tensor_scan(
                STATE[:, 2, k2 + 1:k2 + L + 1],
                AEXP[:, w2 % VD, L:2 * L],
                STATE[:, 3, k2 + 1:k2 + L + 1],
                STATE[:, 2, k2:k2 + 1], Alu.mult, Alu.add)

        # ---- V slabs for wave wv = tw - 1 ----
        if 0 <= wv < limit:
            kv, kvp = (wv % WD) * SL, ((wv - 1) % WD) * SL
            # D[t] = W[t-1] - W[t]  (elem0 = carry guard - W[0], correct)
            nc.gpsimd.tensor_tensor(out=Dring[:, wv % 2, :],
                                    in0=STATE[:, 0, kv:kv + L],
                                    in1=STATE[:, 0, kv + 1:kv + L + 1],
                                    op=Alu.subtract)
            # Aarg|AOarg = (em|blk) + D
            nc.gpsimd.tensor_tensor(
                out=EXPARG[:, wv % VD, 0:2 * L].rearrange("p (a b) -> p a b", a=2),
                in0=emplane[:, wv::(NW - wv), :],
                in1=Dring[:, wv % 2, :].unsqueeze(1).broadcast_to([128, 2, L]),
                op=Alu.add)
            # DLarg = Wprev[t-1] - W[t-1]  (elem0 = guard' - guard, correct)
            nc.gpsimd.tensor_tensor(
                out=EXPARG[:, wv % VD, 2 * L:3 * L],
                in0=STATE[:, 0, kvp:kvp + L],
                in1=STATE[:, 0, kv:kv + L], op=Alu.subtract)
            nc.scalar.activation(out=AEXP[:, wv % VD, 0:2 * L],
                                 in_=EXPARG[:, wv % VD, 0:2 * L], func=Act.Exp)
            nc.scalar.activation(out=AEXP[:, wv % VD, 2 * L:3 * L],
                                 in_=EXPARG[:, wv % VD, 2 * L:3 * L], func=Act.Exp,
                                 bias=c2s[:, wv:wv + 1])

        # ---- E readouts: wave w2 ----
        if tw >= LAG:
            for (g, p, elem) in rd_by_wave_P.get(w2, ()):
                q = (p // 32) * 32
                nc.gpsimd.tensor_tensor(
                    out=outPs[q:q + 32, g:g + 1],
                    in0=STATE[q:q + 32, 3, k2 + 1 + elem:k2 + 2 + elem],
                    in1=zeros1[q:q + 32, 0:1], op=Alu.add)

        # ---- FEED for E wave nw2 = w2+1 = DL[nw2] * q[nw2-1] ring ----
        # AEXP[nw2] was produced by this trace's slab block (wv == nw2);
        # emitted AFTER the q scan so Tile sees the fresh q values.
        nw2 = tw - LAG + 1
        if 0 <= nw2 < limit:
            kq = ((nw2 - 1 + LAG) % WD) * SL
            nc.gpsimd.tensor_tensor(
                out=FEEDr[:, nw2 % 2, :],
                in0=AEXP[:, nw2 % VD, 2 * L:3 * L],
                in1=STATE[:, 2, kq:kq + L],
                op=Alu.mult)

        if tw < limit:
            for (g, p, elem) in rd_by_wave_W.get(w, ()):
                q = (p // 32) * 32
                nc.gpsimd.tensor_tensor(
                    out=outWs[q:q + 32, g:g + 1],
                    in0=STATE[q:q + 32, 0, k + 1 + elem:k + 2 + elem],
                    in1=zeros1[q:q + 32, 0:1], op=Alu.add)

        # ---- carry matmul two traces ahead; emitted AFTER this trace's
        # W/q scans so it reads their fresh ring tails (needed for SLAG=2).
        if tw + 2 < limit + LAG:
            emit_mm(tw + 2)

        # ---- carry-init ACT for E wave tw-LAG+2 (consumes the matmul
        # above; lands 2 traces before its pE/q scans read it) ----
        iw2 = tw - LAG + 2
        if 0 <= iw2 < limit:
            k2n = ((iw2 + LAG) % WD) * SL
            # q guard (row 2) and pE init (row 3) = carry*exp(-jmp) + init
            nc.scalar.activation(
                out=STATE[:, 2:4, k2n:k2n + 1].rearrange("p a b -> p (a b)"),
                in_=CAR4[:, (iw2 + LAG) % 4, 2:4],
                func=Act.Identity,
                bias=initPw[:, iw2:iw2 + 1],
                scale=ejmpn[:, iw2:iw2 + 1])

        # ---- V guard copy for wave tw+2 (W + VE carries -> SBUF guards) --
        if tw + 2 < limit:
            k1 = ((tw + 2) % WD) * SL
            nc.vector.tensor_copy(
                out=STATE[:, 0:2, k1:k1 + 1].rearrange("p a b -> p (a b)"),
                in_=CAR4[:, (tw + 2) % 4, 0:2])

    nc.sync.dma_start(out=outs["outW"], in_=outWs)
    nc.sync.dma_start(out=outs["outP"], in_=outPs)
    ctx.close()


def _build_program(rcells):
    import concourse.bacc as bacc
    import concourse.tile as tile_mod
    from concourse import mybir

    nc = bacc.Bacc("TRN2", target_bir_lowering=False, debug=False,
                   num_devices=NCORES)
    f32 = mybir.dt.float32
    ins = {
        "emplane": nc.declare_dram_parameter("emplane", [128, (NW + 1) * L], f32,
                                             isOutput=False).ap(),
        "c2s": nc.declare_dram_parameter("c2s", [128, NW], f32, isOutput=False).ap(),
        "ejmpn": nc.declare_dram_parameter("ejmpn", [128, NW], f32, isOutput=False).ap(),
        "initPw": nc.declare_dram_parameter("initPw", [128, NW], f32, isOutput=False).ap(),
        "shift8": nc.declare_dram_parameter("shift8", [128, 128], f32, isOutput=False).ap(),
    }
    outs = {
        "outW": nc.declare_dram_parameter("outW", [128, B], f32, isOutput=True).ap(),
        "outP": nc.declare_dram_parameter("outP", [128, B], f32, isOutput=True).ap(),
    }
    with tile_mod.TileContext(nc) as tc:
        build_kernel_body(tc, outs, ins, rcells)
    nc.compile()
    return nc


def kernel(prediction, target, pred_lens, target_lens):
    sys.path.insert(0, "/opt/trn_rl_repo")
    from concourse.bass_utils import run_bass_kernel_spmd

    pl = np.asarray(pred_lens).astype(np.int64)
    tl = np.asarray(target_lens).astype(np.int64)
    rcells = readout_cells(pl, tl)
    in_maps = host_prep(prediction, target)

    nc = _build_program(rcells)
    res = run_bass_kernel_spmd(nc, in_maps, list(range(NCORES)))
    global LAST_RESULTS
    LAST_RESULTS = res

    total = 0.0
    for (g, u_e, t_b, s_b, p, wave, elem) in rcells:
        c = g // BC
        pv = np.float64(res.results[c]["outP"][p, g])
        wv = np.float64(res.results[c]["outW"][p, g])
        total += np.log(pv) + wv + float(_rho(s_b, u_e))
    return np.float32(total)
